# revision 1
# baseline (speedup 1.0000x reference)
"""Llama GQA attention layer (S=2048, H=4096, 32 q heads / 8 kv heads, D=128)
on 8 Trainium2 NeuronCores.

Strategy:
  - Tensor-parallel by heads: core c owns q-heads 4c..4c+3 and kv-head c.
    Wqkv is column-sharded on the host into a per-core [4096, 768] slab
    (512 q cols | 128 k cols | 128 v cols), cast to bf16.
  - hidden_states is shipped pre-transposed ([H, S], bf16) so the QKV
    matmul needs no on-device transpose; RoPE is applied at PSUM-evict
    using host-built cos/sin tables ([128, S], f32).
  - Attention is computed per head in "scores-transposed" layout
    (k on partitions, q on free dim): sT = K^T.T @ Q^T, exp on ACT,
    causal mask via a sliding 0/1 mask multiply, PV and the softmax
    denominator both accumulate in PSUM via matmuls (ones-column trick),
    normalization fused into the PSUM evict.
  - The per-core attention outputs oT [512, 2048] are re-sharded from
    head-parallel to token-parallel with a single small AllToAll
    (bf16, 2.1 MB/core) instead of the 33 MB AllReduce a row-sharded
    o_proj would need.
  - Each core then computes its 256 output rows against the FULL Wo
    (bf16, streamed from HBM), and the host concatenates row shards.
"""
import sys

sys.path.insert(0, "/opt/trn_rl_repo")

from contextlib import ExitStack

import numpy as np

import concourse.bass as bass
import concourse.mybir as mybir
import concourse.tile as tile
from concourse import bacc
from concourse.bass_utils import run_bass_kernel_spmd
from concourse.masks import make_identity

BF16 = mybir.dt.bfloat16
F32 = mybir.dt.float32
FP8 = mybir.dt.float8e4
NPBF16 = mybir.dt.np(BF16)
NPFP8 = mybir.dt.np(FP8)
FP8_SCALE = 64.0

S = 2048          # sequence length
H = 4096          # hidden dim
D = 128           # head dim
NCORES = 8
HPC = 4           # q heads per core
QC = HPC * D      # 512 q cols per core
QKVC = QC + 2 * D  # 768 qkv cols per core
TB = 512          # token block (matmul free dim)
NTB = S // TB     # 4
NKT = H // 128    # 32 contraction tiles
TPC = S // NCORES  # 256 output tokens per core
SCALE = float(D) ** -0.5


def _build_nc(iters=1, nphases=4, attn_heads=HPC, wo_ncb=None,
              skip_coll=False, wo2d=True, qk8=True):
    nc = bacc.Bacc("TRN2", target_bir_lowering=False, debug=False,
                   num_devices=NCORES)

    if qk8:
        hsT = nc.dram_tensor("hsT", [H, S], FP8, kind="ExternalInput").ap()
        hsv = nc.dram_tensor("hsv", [H, S], BF16, kind="ExternalInput").ap()
        wqkv = nc.dram_tensor("wqkv", [H, QC + D], FP8, kind="ExternalInput").ap()
        wqv = nc.dram_tensor("wqv", [H, D], BF16, kind="ExternalInput").ap()
    else:
        hsT = nc.dram_tensor("hsT", [H, S], BF16, kind="ExternalInput").ap()
        hsv = hsT
        wqkv = nc.dram_tensor("wqkv", [H, QKVC], BF16, kind="ExternalInput").ap()
        wqv = None
    wo_cols = H // 2 if wo2d else H
    wo = nc.dram_tensor("wo", [H, wo_cols], BF16, kind="ExternalInput").ap()
    cos2 = nc.dram_tensor("cos2", [D, S], F32, kind="ExternalInput").ap()
    sin2 = nc.dram_tensor("sin2", [D, S], F32, kind="ExternalInput").ap()
    pmask = nc.dram_tensor("pmask", [128, 1280], BF16, kind="ExternalInput").ap()
    out_rows = 2 * TPC if wo2d else TPC
    out = nc.dram_tensor("out", [out_rows, wo_cols], F32,
                         kind="ExternalOutput").ap()

    with tile.TileContext(nc) as tc:
        for _ in range(iters):
            with ExitStack() as ctx:
                _emit(ctx, tc, hsT, hsv, wqkv, wqv, wo, cos2, sin2, pmask, out,
                      nphases, attn_heads, wo_ncb, skip_coll, wo2d, qk8)
    nc.compile()
    return nc


def _emit(ctx, tc, hsT, hsv, wqkv, wqv, wo, cos2, sin2, pmask, out, nphases=4,
          attn_heads=HPC, wo_ncb=None, skip_coll=False, wo2d=True, qk8=True):
    nc = tc.nc
    tgrp = 2 * TPC if wo2d else TPC      # tokens this core projects
    wo_cols = H // 2 if wo2d else H
    if wo_ncb is None:
        wo_ncb = wo_cols // TB

    const = ctx.enter_context(tc.tile_pool(name="const", bufs=1))
    # Wqkv shard resident; q/k cols possibly fp8, v cols bf16; chunked DMAs
    qk_cols = QC + D if qk8 else QKVC
    wq_sb = const.tile([128, NKT, qk_cols], FP8 if qk8 else BF16)
    wq_r = wqkv.rearrange("(kt p) c -> p kt c", p=128)
    for wc in range(4):
        nc.sync.dma_start(out=wq_sb[:, wc * 8:(wc + 1) * 8, :],
                          in_=wq_r[:, wc * 8:(wc + 1) * 8, :])
    if qk8:
        wqv_sb = const.tile([128, NKT, D], BF16)
        nc.sync.dma_start(out=wqv_sb[:], in_=wqv.rearrange("(kt p) c -> p kt c", p=128))
    cos_sb = const.tile([128, S], F32)
    nc.sync.dma_start(out=cos_sb[:], in_=cos2)
    sin_sb = const.tile([128, S], F32)
    nc.sync.dma_start(out=sin_sb[:], in_=sin2)
    mask_sb = const.tile([128, 1280], BF16)
    nc.sync.dma_start(out=mask_sb[:], in_=pmask)
    ones_sb = const.tile([128, 128], BF16)
    nc.gpsimd.memset(ones_sb[:], 1.0)
    ident_sb = const.tile([128, 128], BF16)
    make_identity(nc, ident_sb[:])

    # persistent activations (released before the Wo phase)
    acts_ctx = ExitStack()
    acts = acts_ctx.enter_context(tc.tile_pool(name="acts", bufs=1))
    qT = [acts.tile([128, S], BF16, name=f"qT{h}") for h in range(HPC)]
    kT = acts.tile([128, S], BF16)
    vS = acts.tile([128, 16 * 128], BF16)   # v token-major: [tok%128, (tokblk, d)]
    oT = [acts.tile([128, S], BF16, name=f"oT{h}") for h in range(HPC)]

    qkv_ctx = ExitStack()
    hs_pool = qkv_ctx.enter_context(tc.tile_pool(name="hs", bufs=2))
    qkv_psum = qkv_ctx.enter_context(tc.tile_pool(name="qkvps", bufs=3, space="PSUM"))
    ev_pool = qkv_ctx.enter_context(tc.tile_pool(name="ev", bufs=2))
    tp_psum = qkv_ctx.enter_context(tc.tile_pool(name="tpps", bufs=2, space="PSUM"))

    # ---- QKV projection + RoPE + V transpose ----
    for tb in range(NTB):
        hs_sb = hs_pool.tile([128, NKT, TB], FP8 if qk8 else BF16, tag="hs8")
        hs_r = hsT[:, tb * TB:(tb + 1) * TB].rearrange("(kt p) t -> p kt t", p=128)
        for hc in range(4):
            nc.sync.dma_start(out=hs_sb[:, hc * 8:(hc + 1) * 8, :],
                              in_=hs_r[:, hc * 8:(hc + 1) * 8, :])
        if qk8:
            hsv_sb = hs_pool.tile([128, NKT, TB], BF16, tag="hsv", bufs=1)
            hsv_r = hsv[:, tb * TB:(tb + 1) * TB].rearrange("(kt p) t -> p kt t",
                                                            p=128)
            for hc in range(4):
                nc.sync.dma_start(out=hsv_sb[:, hc * 8:(hc + 1) * 8, :],
                                  in_=hsv_r[:, hc * 8:(hc + 1) * 8, :])
        else:
            hsv_sb = hs_sb
        for cb in range(6):
            ps = qkv_psum.tile([128, TB], F32)
            if qk8 and cb < 5:
                for kt2 in range(NKT // 2):
                    nc.tensor.matmul(
                        ps[:],
                        lhsT=wq_sb[:, 2 * kt2:2 * kt2 + 2, cb * 128:(cb + 1) * 128],
                        rhs=hs_sb[:, 2 * kt2:2 * kt2 + 2, :],
                        start=(kt2 == 0), stop=(kt2 == NKT // 2 - 1),
                        perf_mode=mybir.MatmulPerfMode.DoubleRow,
                    )
            elif qk8:
                for kt in range(NKT):
                    nc.tensor.matmul(
                        ps[:],
                        lhsT=wqv_sb[:, kt, :],
                        rhs=hsv_sb[:, kt, :],
                        start=(kt == 0), stop=(kt == NKT - 1),
                    )
            else:
                for kt in range(NKT):
                    nc.tensor.matmul(
                        ps[:],
                        lhsT=wq_sb[:, kt, cb * 128:(cb + 1) * 128],
                        rhs=hs_sb[:, kt, :],
                        start=(kt == 0), stop=(kt == NKT - 1),
                    )
            if cb < 5:
                # q head cb (cb<4) or k (cb==4): RoPE at evict
                s32 = ev_pool.tile([128, TB], F32, tag="s32")
                nc.scalar.copy(out=s32[:], in_=ps[:])
                qs = ev_pool.tile([128, TB], F32, tag="qs")
                nc.sync.dma_start(out=qs[0:64, :], in_=s32[64:128, :])
                nc.sync.dma_start(out=qs[64:128, :], in_=s32[0:64, :])
                t1 = ev_pool.tile([128, TB], F32, tag="t1")
                csl = slice(tb * TB, (tb + 1) * TB)
                nc.vector.tensor_mul(out=t1[:], in0=s32[:], in1=cos_sb[:, csl])
                t2 = ev_pool.tile([128, TB], F32, tag="t2")
                nc.vector.tensor_mul(out=t2[:], in0=qs[:], in1=sin_sb[:, csl])
                dst = qT[cb] if cb < HPC else kT
                nc.vector.tensor_sub(out=dst[:, csl], in0=t1[:], in1=t2[:])
            else:
                # v: evict bf16 then transpose [128,128] chunks to token-major
                vT = ev_pool.tile([128, TB], BF16, tag="vT")
                nc.scalar.copy(out=vT[:], in_=ps[:])
                for i in range(TB // 128):
                    tp = tp_psum.tile([128, 128], BF16)
                    nc.tensor.transpose(tp[:], vT[:, i * 128:(i + 1) * 128],
                                        ident_sb[:])
                    st = tb * 4 + i
                    nc.scalar.copy(out=vS[:, st * 128:(st + 1) * 128], in_=tp[:])

    qkv_ctx.close()
    if nphases < 2:
        # timing bisection: dump a QKV product so nothing is dead-code'd
        st = ctx.enter_context(tc.tile_pool(name="stg", bufs=2))
        for h in range(2):
            sg = st.tile([128, TPC], F32, tag="sg")
            nc.scalar.copy(out=sg[:], in_=qT[h][:, :TPC])
            nc.sync.dma_start(out=out[h * 128:(h + 1) * 128, :TPC], in_=sg[:])
        return

    # ---- attention (per head, scores-transposed flash style) ----
    dram = ctx.enter_context(tc.tile_pool(name="dram", bufs=1, space="DRAM"))
    a2a_ins = [dram.tile([NCORES, QC // 2, tgrp], BF16, name=f"a2ai{m}")
               for m in range(2)]
    a2a_outs = [dram.tile([NCORES, QC // 2, tgrp], BF16, name=f"a2ao{m}")
                for m in range(2)]

    at_ctx = ExitStack()
    at_psum = at_ctx.enter_context(tc.tile_pool(name="atps", bufs=3, space="PSUM"))
    acc_psum = at_ctx.enter_context(tc.tile_pool(name="accps", bufs=2, space="PSUM"))
    pr_pool = at_ctx.enter_context(tc.tile_pool(name="pr", bufs=3))
    nrm_pool = at_ctx.enter_context(tc.tile_pool(name="nrm", bufs=2))

    for h in range(attn_heads):
        for qt in range(NTB):
            o_ps = acc_psum.tile([128, TB], F32, tag="o")
            den_ps = acc_psum.tile([128, TB], F32, tag="den")
            nkt2 = 4 * qt + 4

            def emit_pv(pr, kt2):
                nc.tensor.matmul(
                    o_ps[:], lhsT=vS[:, kt2 * 128:(kt2 + 1) * 128], rhs=pr[:],
                    start=(kt2 == 0), stop=(kt2 == nkt2 - 1), skip_group_check=True,
                )
                nc.tensor.matmul(
                    den_ps[:], lhsT=ones_sb[:], rhs=pr[:],
                    start=(kt2 == 0), stop=(kt2 == nkt2 - 1), skip_group_check=True,
                )

            # 1-deep software pipeline: QK(k+1) issues on PE before PV/den(k),
            # so the ACT exp of tile k hides behind tensor work.
            pending = None
            for kt2 in range(nkt2):
                s_ps = at_psum.tile([128, TB], F32)
                nc.tensor.matmul(
                    s_ps[:],
                    lhsT=kT[:, kt2 * 128:(kt2 + 1) * 128],
                    rhs=qT[h][:, qt * TB:(qt + 1) * TB],
                    start=True, stop=True,
                )
                pr = pr_pool.tile([128, TB], BF16)
                eff_scale = SCALE / (FP8_SCALE ** 4) if qk8 else SCALE
                nc.scalar.activation(pr[:], s_ps[:],
                                     mybir.ActivationFunctionType.Exp,
                                     scale=eff_scale)
                o = qt * TB - kt2 * 128
                if o <= 384:  # diagonal tile: apply causal mask
                    nc.vector.tensor_mul(
                        out=pr[:], in0=pr[:],
                        in1=mask_sb[:, 384 + o:384 + o + TB],
                    )
                if pending is not None:
                    emit_pv(*pending)
                pending = (pr, kt2)
            emit_pv(*pending)
            rd = nrm_pool.tile([128, TB], F32)
            nc.vector.reciprocal(out=rd[:], in_=den_ps[:])
            nc.vector.tensor_mul(out=oT[h][:, qt * TB:(qt + 1) * TB],
                                 in0=o_ps[:], in1=rd[:])
        if nphases >= 3:
            # ship this head's A2A input slices as soon as the head is done;
            # fire each half-collective when its two heads are complete
            for j in range(NCORES):
                g = j % 4 if wo2d else j
                nc.sync.dma_start(
                    out=a2a_ins[h // 2][j, (h % 2) * 128:(h % 2 + 1) * 128, :],
                    in_=oT[h][:, g * tgrp:(g + 1) * tgrp],
                )
            if h % 2 == 1 and not skip_coll:
                nc.gpsimd.collective_compute(
                    "AllToAll", mybir.AluOpType.bypass,
                    replica_groups=[list(range(NCORES))],
                    ins=[a2a_ins[h // 2].opt()], outs=[a2a_outs[h // 2].opt()],
                )

    at_ctx.close()
    if nphases < 3:
        st = ctx.enter_context(tc.tile_pool(name="stg", bufs=2))
        for h in range(2):
            sg = st.tile([128, TPC], F32, tag="sg")
            nc.scalar.copy(out=sg[:], in_=oT[h][:, :TPC])
            nc.sync.dma_start(out=out[h * 128:(h + 1) * 128, :TPC], in_=sg[:])
        return

    # ---- AllToAll handled inside the attention loop (two halves) ----
    if skip_coll:
        a2a_outs = a2a_ins

    # ---- output projection ----
    acts_ctx.close()
    olp = ctx.enter_context(tc.tile_pool(name="olp", bufs=1))
    oL = olp.tile([128, NCORES, 4, tgrp], BF16)
    for m in range(2):
        src_r = a2a_outs[m].rearrange("g (k2 p) t -> p g k2 t", p=128)
        for k2 in range(2):
            nc.sync.dma_start(
                out=oL[:, :, m * 2 + k2, :],
                in_=src_r[:, :, k2, :],
            )
    if nphases < 4:
        st = ctx.enter_context(tc.tile_pool(name="stg", bufs=2))
        for h in range(2):
            sg = st.tile([128, TPC], F32, tag="sg")
            nc.scalar.copy(out=sg[:], in_=oL[:, 0, h, :TPC])
            nc.sync.dma_start(out=out[h * 128:(h + 1) * 128, :TPC], in_=sg[:])
        return
    nth = tgrp // 128
    wo_pool = ctx.enter_context(tc.tile_pool(name="wo", bufs=2))
    out_psum = ctx.enter_context(
        tc.tile_pool(name="ops", bufs=2 if nth <= 2 else 1, space="PSUM"))
    res_pool = ctx.enter_context(tc.tile_pool(name="res", bufs=3))
    for ncb in range(wo_ncb):
        # one 4MB DMA per output-column block: wo[:, ncb*512:+512] as [p, kt, n]
        wt = wo_pool.tile([128, NKT, TB], BF16)
        nc.sync.dma_start(
            out=wt[:],
            in_=wo[:, ncb * TB:(ncb + 1) * TB].rearrange("(kt p) n -> p kt n", p=128),
        )
        pss = [out_psum.tile([128, TB], F32, tag=f"po{th}", name=f"po{th}")
               for th in range(nth)]
        for kt in range(NKT):
            for th in range(nth):
                nc.tensor.matmul(
                    pss[th][:],
                    lhsT=oL[:, kt // 4, kt % 4, th * 128:(th + 1) * 128],
                    rhs=wt[:, kt, :],
                    start=(kt == 0), stop=(kt == NKT - 1), skip_group_check=True,
                )
        for th in range(nth):
            rs = res_pool.tile([128, TB], F32)
            nc.scalar.copy(out=rs[:], in_=pss[th][:])
            nc.sync.dma_start(
                out=out[th * 128:(th + 1) * 128, ncb * TB:(ncb + 1) * TB],
                in_=rs[:],
            )


_NC_CACHE = {}


def _get_nc():
    if "nc" not in _NC_CACHE:
        _NC_CACHE["nc"] = _build_nc()
    return _NC_CACHE["nc"]


def _host_prep(positions, hidden_states, Wqkv, Wo, wo2d=True, qk8=True):
    positions = np.asarray(positions)
    hidden_states = np.asarray(hidden_states, dtype=np.float32)
    Wqkv = np.asarray(Wqkv, dtype=np.float32)
    Wo = np.asarray(Wo, dtype=np.float32)

    hsT_f32 = np.ascontiguousarray(hidden_states.T)
    hsT = hsT_f32.astype(NPBF16)
    hs8 = (hsT_f32 * FP8_SCALE).astype(NPFP8)
    wo_bf = Wo.astype(NPBF16)
    if wo2d:
        wo_halves = [np.ascontiguousarray(wo_bf[:, :H // 2]),
                     np.ascontiguousarray(wo_bf[:, H // 2:])]

    q_size = 32 * D
    wqkv_shards = []
    for c in range(NCORES):
        qcols = Wqkv[:, c * QC:(c + 1) * QC]
        kcols = Wqkv[:, q_size + c * D:q_size + (c + 1) * D]
        vcols = Wqkv[:, q_size + 8 * D + c * D:q_size + 8 * D + (c + 1) * D]
        wqkv_shards.append(
            np.ascontiguousarray(np.concatenate([qcols, kcols, vcols], axis=1))
            .astype(NPBF16)
        )

    half = D // 2
    inv_freq = (1.0 / (10000.0 ** (np.arange(0, half, dtype=np.float32) / half))
                ).astype(np.float32)
    ang = positions.astype(np.float32)[:, None] * inv_freq[None, :]  # [S, 64]
    cosT = np.cos(ang).astype(np.float32).T  # [64, S]
    sinT = np.sin(ang).astype(np.float32).T
    cos2 = np.ascontiguousarray(np.vstack([cosT, cosT]))
    sin2 = np.ascontiguousarray(np.vstack([sinT, -sinT]))

    pm = (np.arange(128)[:, None] <= (np.arange(1280)[None, :] - 384))
    pmask = pm.astype(NPBF16)

    if qk8:
        common = {"hsT": hs8, "hsv": hsT, "cos2": cos2, "sin2": sin2,
                  "pmask": pmask}
        maps = []
        for c in range(NCORES):
            qcols = Wqkv[:, c * QC:(c + 1) * QC]
            kcols = Wqkv[:, q_size + c * D:q_size + (c + 1) * D]
            vcols = Wqkv[:, q_size + 8 * D + c * D:q_size + 8 * D + (c + 1) * D]
            wq8 = np.ascontiguousarray(
                np.concatenate([qcols, kcols], axis=1) * FP8_SCALE).astype(NPFP8)
            wv = np.ascontiguousarray(vcols).astype(NPBF16)
            m = dict(common, wqkv=wq8, wqv=wv,
                     wo=wo_halves[c // 4] if wo2d else wo_bf)
            maps.append(m)
        return maps
    common = {"hsT": hsT, "cos2": cos2, "sin2": sin2, "pmask": pmask}
    if wo2d:
        return [dict(common, wqkv=wqkv_shards[c], wo=wo_halves[c // 4])
                for c in range(NCORES)]
    return [dict(common, wqkv=wqkv_shards[c], wo=wo_bf) for c in range(NCORES)]


def _assemble(outs):
    full = np.empty((S, H), np.float32)
    for c in range(NCORES):
        g, ch = c % 4, c // 4
        full[g * 512:(g + 1) * 512, ch * (H // 2):(ch + 1) * (H // 2)] = outs[c]
    return full


def kernel(positions, hidden_states, Wqkv, Wo):
    in_maps = _host_prep(positions, hidden_states, Wqkv, Wo, wo2d=True)
    nc = _get_nc()
    res = run_bass_kernel_spmd(nc, in_maps, list(range(NCORES)))
    return _assemble([res.results[c]["out"] for c in range(NCORES)])



# revision 12
# speedup vs baseline: 1.7184x; 1.7184x over previous
"""Llama GQA attention layer (S=2048, H=4096, 32 q heads / 8 kv heads, D=128)
on 8 Trainium2 NeuronCores.

Strategy:
  - Tensor-parallel by heads: core c owns q-heads 4c..4c+3 and kv-head c.
    Wqkv is column-sharded on the host into a per-core [4096, 768] slab
    (512 q cols | 128 k cols | 128 v cols), cast to bf16.
  - hidden_states is shipped pre-transposed ([H, S], bf16) so the QKV
    matmul needs no on-device transpose; RoPE is applied at PSUM-evict
    using host-built cos/sin tables ([128, S], f32).
  - Attention is computed per head in "scores-transposed" layout
    (k on partitions, q on free dim): sT = K^T.T @ Q^T, exp on ACT,
    causal mask via a sliding 0/1 mask multiply, PV and the softmax
    denominator both accumulate in PSUM via matmuls (ones-column trick),
    normalization fused into the PSUM evict.
  - The per-core attention outputs oT [512, 2048] are re-sharded from
    head-parallel to token-parallel with four small per-head AllToAlls
    (bf16, 1 MB/core each) fired as each head completes, instead of the
    33 MB AllReduce a row-sharded o_proj would need; o_proj's first
    column block contracts chunk-major so PE work starts while the last
    collectives land.
  - Each core then computes its 512 output rows against half of Wo
    (bf16, streamed from HBM), and the host assembles the 2-D grid.
"""
import sys

sys.path.insert(0, "/opt/trn_rl_repo")

from contextlib import ExitStack

import numpy as np

import concourse.bass as bass
import concourse.mybir as mybir
import concourse.tile as tile
from concourse import bacc
from concourse.bass_utils import run_bass_kernel_spmd
from concourse.masks import make_identity

BF16 = mybir.dt.bfloat16
F32 = mybir.dt.float32
FP8 = mybir.dt.float8e4
NPBF16 = mybir.dt.np(BF16)
NPFP8 = mybir.dt.np(FP8)
FP8_SCALE = 64.0

S = 2048          # sequence length
H = 4096          # hidden dim
D = 128           # head dim
NCORES = 8
HPC = 4           # q heads per core
QC = HPC * D      # 512 q cols per core
QKVC = QC + 2 * D  # 768 qkv cols per core
TB = 512          # token block (matmul free dim)
NTB = S // TB     # 4
NKT = H // 128    # 32 contraction tiles
TPC = S // NCORES  # 256 output tokens per core
SCALE = float(D) ** -0.5


def _build_nc(iters=1, nphases=4, attn_heads=HPC, wo_ncb=None,
              skip_coll=False, wo2d=True, qk8=True):
    nc = bacc.Bacc("TRN2", target_bir_lowering=False, debug=False,
                   num_devices=NCORES)

    if qk8:
        hsT = nc.dram_tensor("hsT", [H, S], FP8, kind="ExternalInput").ap()
        hsv = nc.dram_tensor("hsv", [H, S], BF16, kind="ExternalInput").ap()
        wqkv = nc.dram_tensor("wqkv", [H, QC + D], FP8, kind="ExternalInput").ap()
        wqv = nc.dram_tensor("wqv", [H, D], BF16, kind="ExternalInput").ap()
    else:
        hsT = nc.dram_tensor("hsT", [H, S], BF16, kind="ExternalInput").ap()
        hsv = hsT
        wqkv = nc.dram_tensor("wqkv", [H, QKVC], BF16, kind="ExternalInput").ap()
        wqv = None
    wo_cols = H // 2 if wo2d else H
    wo = nc.dram_tensor("wo", [H, wo_cols], BF16, kind="ExternalInput").ap()
    cos2 = nc.dram_tensor("cos2", [D, S], F32, kind="ExternalInput").ap()
    sin2 = nc.dram_tensor("sin2", [D, S], F32, kind="ExternalInput").ap()
    pmask = nc.dram_tensor("pmask", [128, 1280], BF16, kind="ExternalInput").ap()
    out_rows = 2 * TPC if wo2d else TPC
    out = nc.dram_tensor("out", [out_rows, wo_cols], F32,
                         kind="ExternalOutput").ap()

    with tile.TileContext(nc) as tc:
        for _ in range(iters):
            with ExitStack() as ctx:
                _emit(ctx, tc, hsT, hsv, wqkv, wqv, wo, cos2, sin2, pmask, out,
                      nphases, attn_heads, wo_ncb, skip_coll, wo2d, qk8)
    nc.compile()
    return nc


def _emit(ctx, tc, hsT, hsv, wqkv, wqv, wo, cos2, sin2, pmask, out, nphases=4,
          attn_heads=HPC, wo_ncb=None, skip_coll=False, wo2d=True, qk8=True):
    nc = tc.nc
    tgrp = 2 * TPC if wo2d else TPC      # tokens this core projects
    wo_cols = H // 2 if wo2d else H
    if wo_ncb is None:
        wo_ncb = wo_cols // TB

    const = ctx.enter_context(tc.tile_pool(name="const", bufs=1))
    # Wqkv shard resident; q/k cols possibly fp8, v cols bf16; chunked DMAs
    qk_cols = QC + D if qk8 else QKVC
    wq_sb = const.tile([128, NKT, qk_cols], FP8 if qk8 else BF16)
    wq_r = wqkv.rearrange("(kt p) c -> p kt c", p=128)
    for wc in range(4):
        nc.sync.dma_start(out=wq_sb[:, wc * 8:(wc + 1) * 8, :],
                          in_=wq_r[:, wc * 8:(wc + 1) * 8, :])
    if qk8:
        wqv_sb = const.tile([128, NKT, D], BF16)
        nc.sync.dma_start(out=wqv_sb[:], in_=wqv.rearrange("(kt p) c -> p kt c", p=128))
    cos_sb = const.tile([128, S], F32)
    nc.sync.dma_start(out=cos_sb[:], in_=cos2)
    sin_sb = const.tile([128, S], F32)
    nc.sync.dma_start(out=sin_sb[:], in_=sin2)
    mask_sb = const.tile([128, 1280], BF16)
    nc.sync.dma_start(out=mask_sb[:], in_=pmask)
    ones_sb = const.tile([128, 128], BF16)
    nc.gpsimd.memset(ones_sb[:], 1.0)
    ident_sb = const.tile([128, 128], BF16)
    make_identity(nc, ident_sb[:])

    # persistent activations (live for the whole iteration; SBUF still fits
    # with the o_proj working set, so no early release)
    acts = ctx.enter_context(tc.tile_pool(name="acts", bufs=1))
    qT = [acts.tile([128, S], BF16, name=f"qT{h}") for h in range(HPC)]
    kT = acts.tile([128, S], BF16)
    vS = acts.tile([128, 16 * 128], BF16)   # v token-major: [tok%128, (tokblk, d)]
    oT = [acts.tile([128, S], BF16, name=f"oT{h}") for h in range(HPC)]

    qkv_ctx = ExitStack()
    hs_pool = qkv_ctx.enter_context(tc.tile_pool(name="hs", bufs=2))
    qkv_psum = qkv_ctx.enter_context(tc.tile_pool(name="qkvps", bufs=3, space="PSUM"))
    ev_pool = qkv_ctx.enter_context(tc.tile_pool(name="ev", bufs=2))
    tp_psum = qkv_ctx.enter_context(tc.tile_pool(name="tpps", bufs=2, space="PSUM"))

    # ---- QKV projection + RoPE + V transpose ----
    for tb in range(NTB):
        hs_sb = hs_pool.tile([128, NKT, TB], FP8 if qk8 else BF16, tag="hs8")
        hs_r = hsT[:, tb * TB:(tb + 1) * TB].rearrange("(kt p) t -> p kt t", p=128)
        for hc in range(4):
            nc.sync.dma_start(out=hs_sb[:, hc * 8:(hc + 1) * 8, :],
                              in_=hs_r[:, hc * 8:(hc + 1) * 8, :])
        if qk8:
            hsv_sb = hs_pool.tile([128, NKT, TB], BF16, tag="hsv", bufs=1)
            hsv_r = hsv[:, tb * TB:(tb + 1) * TB].rearrange("(kt p) t -> p kt t",
                                                            p=128)
            for hc in range(4):
                nc.sync.dma_start(out=hsv_sb[:, hc * 8:(hc + 1) * 8, :],
                                  in_=hsv_r[:, hc * 8:(hc + 1) * 8, :])
        else:
            hsv_sb = hs_sb
        for cb in range(6):
            ps = qkv_psum.tile([128, TB], F32)
            if qk8 and cb < 5:
                for kt2 in range(NKT // 2):
                    nc.tensor.matmul(
                        ps[:],
                        lhsT=wq_sb[:, 2 * kt2:2 * kt2 + 2, cb * 128:(cb + 1) * 128],
                        rhs=hs_sb[:, 2 * kt2:2 * kt2 + 2, :],
                        start=(kt2 == 0), stop=(kt2 == NKT // 2 - 1),
                        perf_mode=mybir.MatmulPerfMode.DoubleRow,
                    )
            elif qk8:
                for kt in range(NKT):
                    nc.tensor.matmul(
                        ps[:],
                        lhsT=wqv_sb[:, kt, :],
                        rhs=hsv_sb[:, kt, :],
                        start=(kt == 0), stop=(kt == NKT - 1),
                    )
            else:
                for kt in range(NKT):
                    nc.tensor.matmul(
                        ps[:],
                        lhsT=wq_sb[:, kt, cb * 128:(cb + 1) * 128],
                        rhs=hs_sb[:, kt, :],
                        start=(kt == 0), stop=(kt == NKT - 1),
                    )
            if cb < 5:
                # q head cb (cb<4) or k (cb==4): RoPE at evict
                s32 = ev_pool.tile([128, TB], F32, tag="s32")
                nc.scalar.copy(out=s32[:], in_=ps[:])
                qs = ev_pool.tile([128, TB], F32, tag="qs")
                nc.sync.dma_start(out=qs[0:64, :], in_=s32[64:128, :])
                nc.sync.dma_start(out=qs[64:128, :], in_=s32[0:64, :])
                t1 = ev_pool.tile([128, TB], F32, tag="t1")
                csl = slice(tb * TB, (tb + 1) * TB)
                nc.vector.tensor_mul(out=t1[:], in0=s32[:], in1=cos_sb[:, csl])
                t2 = ev_pool.tile([128, TB], F32, tag="t2")
                nc.vector.tensor_mul(out=t2[:], in0=qs[:], in1=sin_sb[:, csl])
                dst = qT[cb] if cb < HPC else kT
                nc.vector.tensor_sub(out=dst[:, csl], in0=t1[:], in1=t2[:])
            else:
                # v: evict bf16 then transpose [128,128] chunks to token-major
                vT = ev_pool.tile([128, TB], BF16, tag="vT")
                nc.scalar.copy(out=vT[:], in_=ps[:])
                for i in range(TB // 128):
                    tp = tp_psum.tile([128, 128], BF16)
                    nc.tensor.transpose(tp[:], vT[:, i * 128:(i + 1) * 128],
                                        ident_sb[:])
                    st = tb * 4 + i
                    nc.scalar.copy(out=vS[:, st * 128:(st + 1) * 128], in_=tp[:])

    qkv_ctx.close()
    if nphases < 2:
        # timing bisection: dump a QKV product so nothing is dead-code'd
        with ExitStack() as sctx:
            st = sctx.enter_context(tc.tile_pool(name="stg", bufs=2))
            for h in range(2):
                sg = st.tile([128, TPC], F32, tag="sg")
                nc.scalar.copy(out=sg[:], in_=qT[h][:, :TPC])
                nc.sync.dma_start(out=out[h * 128:(h + 1) * 128, :TPC], in_=sg[:])
        return

    # ---- attention (per head, scores-transposed flash style) ----
    dram = ctx.enter_context(tc.tile_pool(name="dram", bufs=1, space="DRAM"))
    a2a_ins = [dram.tile([NCORES, D, tgrp], BF16, name=f"a2ai{m}")
               for m in range(HPC)]
    a2a_outs = [dram.tile([NCORES, D, tgrp], BF16, name=f"a2ao{m}")
                for m in range(HPC)]

    # o_proj inputs, gathered per head-chunk as soon as its A2A lands.
    # oL[c][p, j, t]: contraction rows of global k-tile 4j+c, tokens t.
    olp = ctx.enter_context(tc.tile_pool(name="olp", bufs=1))
    oL = [olp.tile([128, NCORES, tgrp], BF16, name=f"oL{c}", tag=f"oL{c}")
          for c in range(HPC)]
    # prefetch the first two Wo column blocks during attention (scalar DMA
    # queue, ahead of the collective-gated oL gathers in that FIFO)
    wo_pool = ctx.enter_context(tc.tile_pool(name="wo", bufs=2))
    wts = []
    if nphases >= 4:
        for pf in range(2):
            wt_pf = wo_pool.tile([128, NKT, TB], BF16, tag="wt", name=f"wt{pf}")
            nc.scalar.dma_start(
                out=wt_pf[:],
                in_=wo[:, pf * TB:(pf + 1) * TB].rearrange("(kt p) n -> p kt n",
                                                           p=128),
            )
            wts.append(wt_pf)

    at_ctx = ExitStack()
    at_psum = at_ctx.enter_context(tc.tile_pool(name="atps", bufs=3, space="PSUM"))
    acc_psum = at_ctx.enter_context(tc.tile_pool(name="accps", bufs=2, space="PSUM"))
    pr_pool = at_ctx.enter_context(tc.tile_pool(name="pr", bufs=4))
    nrm_pool = at_ctx.enter_context(tc.tile_pool(name="nrm", bufs=2))

    for h in range(attn_heads):
        for qt in range(NTB):
            o_ps = acc_psum.tile([128, TB], F32, tag="o")
            den_ps = acc_psum.tile([128, TB], F32, tag="den")
            nkt2 = 4 * qt + 4

            def emit_pv(pr, kt2):
                nc.tensor.matmul(
                    o_ps[:], lhsT=vS[:, kt2 * 128:(kt2 + 1) * 128], rhs=pr[:],
                    start=(kt2 == 0), stop=(kt2 == nkt2 - 1), skip_group_check=True,
                )
                nc.tensor.matmul(
                    den_ps[:], lhsT=ones_sb[:], rhs=pr[:],
                    start=(kt2 == 0), stop=(kt2 == nkt2 - 1), skip_group_check=True,
                )

            # 2-deep software pipeline: QK(k+1) and QK(k+2) issue on PE before
            # PV/den(k), so the ACT exp + DVE mask of tile k fully hide
            # behind tensor work even on masked (diagonal) tiles.
            pending = []
            for kt2 in range(nkt2):
                s_ps = at_psum.tile([128, TB], F32)
                nc.tensor.matmul(
                    s_ps[:],
                    lhsT=kT[:, kt2 * 128:(kt2 + 1) * 128],
                    rhs=qT[h][:, qt * TB:(qt + 1) * TB],
                    start=True, stop=True,
                )
                pr = pr_pool.tile([128, TB], BF16)
                eff_scale = SCALE / (FP8_SCALE ** 4) if qk8 else SCALE
                nc.scalar.activation(pr[:], s_ps[:],
                                     mybir.ActivationFunctionType.Exp,
                                     scale=eff_scale)
                o = qt * TB - kt2 * 128
                if o <= 384:  # diagonal tile: apply causal mask
                    nc.vector.tensor_mul(
                        out=pr[:], in0=pr[:],
                        in1=mask_sb[:, 384 + o:384 + o + TB],
                    )
                pending.append((pr, kt2))
                if len(pending) > 2:
                    emit_pv(*pending.pop(0))
            for p_ in pending:
                emit_pv(*p_)
            rd = nrm_pool.tile([128, TB], F32)
            nc.vector.reciprocal(out=rd[:], in_=den_ps[:])
            nc.vector.tensor_mul(out=oT[h][:, qt * TB:(qt + 1) * TB],
                                 in0=o_ps[:], in1=rd[:])
        if nphases >= 3:
            # ship this head's A2A input slices and fire its collective as
            # soon as the head is done; gather the resulting o_proj chunk on
            # the scalar DMA queue (so later heads' staging DMAs on the sync
            # queue are not blocked behind the collective-gated gather).
            for j in range(NCORES):
                g = j % 4 if wo2d else j
                nc.sync.dma_start(
                    out=a2a_ins[h][j, :, :],
                    in_=oT[h][:, g * tgrp:(g + 1) * tgrp],
                )
            if not skip_coll:
                nc.gpsimd.collective_compute(
                    "AllToAll", mybir.AluOpType.bypass,
                    replica_groups=[list(range(NCORES))],
                    ins=[a2a_ins[h].opt()], outs=[a2a_outs[h].opt()],
                )
            nc.scalar.dma_start(
                out=oL[h][:],
                in_=(a2a_ins[h] if skip_coll else a2a_outs[h])
                .rearrange("g p t -> p g t"),
            )

    at_ctx.close()
    if nphases < 3:
        with ExitStack() as sctx:
            st = sctx.enter_context(tc.tile_pool(name="stg", bufs=2))
            for h in range(2):
                sg = st.tile([128, TPC], F32, tag="sg")
                nc.scalar.copy(out=sg[:], in_=oT[h][:, :TPC])
                nc.sync.dma_start(out=out[h * 128:(h + 1) * 128, :TPC], in_=sg[:])
        return

    # ---- output projection ----
    if nphases < 4:
        with ExitStack() as sctx:
            st = sctx.enter_context(tc.tile_pool(name="stg", bufs=2))
            for h in range(2):
                sg = st.tile([128, TPC], F32, tag="sg")
                nc.scalar.copy(out=sg[:], in_=oL[h][:, 0, :TPC])
                nc.sync.dma_start(out=out[h * 128:(h + 1) * 128, :TPC], in_=sg[:])
        return
    nth = tgrp // 128
    out_psum = ctx.enter_context(
        tc.tile_pool(name="ops", bufs=2 if nth <= 2 else 1, space="PSUM"))
    res_pool = ctx.enter_context(tc.tile_pool(name="res", bufs=3))
    for ncb in range(wo_ncb):
        if ncb < len(wts):
            wt = wts[ncb]
        else:
            # one 4MB DMA per output-column block: wo[:, ncb*512:+512]
            wt = wo_pool.tile([128, NKT, TB], BF16, tag="wt")
            nc.scalar.dma_start(
                out=wt[:],
                in_=wo[:, ncb * TB:(ncb + 1) * TB].rearrange("(kt p) n -> p kt n",
                                                             p=128),
            )
        pss = [out_psum.tile([128, TB], F32, tag=f"po{th}", name=f"po{th}")
               for th in range(nth)]
        # chunk-major (c outer): at step c only head-chunk c's A2A is needed,
        # so PE starts on chunk 0 while the later heads' collectives land.
        for c in range(HPC):
            for j in range(NCORES):
                for th in range(nth):
                    nc.tensor.matmul(
                        pss[th][:],
                        lhsT=oL[c][:, j, th * 128:(th + 1) * 128],
                        rhs=wt[:, 4 * j + c, :],
                        start=(c == 0 and j == 0),
                        stop=(c == HPC - 1 and j == NCORES - 1),
                        skip_group_check=True,
                    )
        for th in range(nth):
            rs = res_pool.tile([128, TB], F32)
            nc.scalar.copy(out=rs[:], in_=pss[th][:])
            nc.sync.dma_start(
                out=out[th * 128:(th + 1) * 128, ncb * TB:(ncb + 1) * TB],
                in_=rs[:],
            )


_NC_CACHE = {}


def _get_nc():
    if "nc" not in _NC_CACHE:
        _NC_CACHE["nc"] = _build_nc()
    return _NC_CACHE["nc"]


def _host_prep(positions, hidden_states, Wqkv, Wo, wo2d=True, qk8=True):
    positions = np.asarray(positions)
    hidden_states = np.asarray(hidden_states, dtype=np.float32)
    Wqkv = np.asarray(Wqkv, dtype=np.float32)
    Wo = np.asarray(Wo, dtype=np.float32)

    hsT_f32 = np.ascontiguousarray(hidden_states.T)
    hsT = hsT_f32.astype(NPBF16)
    hs8 = (hsT_f32 * FP8_SCALE).astype(NPFP8)
    wo_bf = Wo.astype(NPBF16)
    if wo2d:
        wo_halves = [np.ascontiguousarray(wo_bf[:, :H // 2]),
                     np.ascontiguousarray(wo_bf[:, H // 2:])]

    q_size = 32 * D
    wqkv_shards = []
    for c in range(NCORES):
        qcols = Wqkv[:, c * QC:(c + 1) * QC]
        kcols = Wqkv[:, q_size + c * D:q_size + (c + 1) * D]
        vcols = Wqkv[:, q_size + 8 * D + c * D:q_size + 8 * D + (c + 1) * D]
        wqkv_shards.append(
            np.ascontiguousarray(np.concatenate([qcols, kcols, vcols], axis=1))
            .astype(NPBF16)
        )

    half = D // 2
    inv_freq = (1.0 / (10000.0 ** (np.arange(0, half, dtype=np.float32) / half))
                ).astype(np.float32)
    ang = positions.astype(np.float32)[:, None] * inv_freq[None, :]  # [S, 64]
    cosT = np.cos(ang).astype(np.float32).T  # [64, S]
    sinT = np.sin(ang).astype(np.float32).T
    cos2 = np.ascontiguousarray(np.vstack([cosT, cosT]))
    sin2 = np.ascontiguousarray(np.vstack([sinT, -sinT]))

    pm = (np.arange(128)[:, None] <= (np.arange(1280)[None, :] - 384))
    pmask = pm.astype(NPBF16)

    if qk8:
        common = {"hsT": hs8, "hsv": hsT, "cos2": cos2, "sin2": sin2,
                  "pmask": pmask}
        maps = []
        for c in range(NCORES):
            qcols = Wqkv[:, c * QC:(c + 1) * QC]
            kcols = Wqkv[:, q_size + c * D:q_size + (c + 1) * D]
            vcols = Wqkv[:, q_size + 8 * D + c * D:q_size + 8 * D + (c + 1) * D]
            wq8 = np.ascontiguousarray(
                np.concatenate([qcols, kcols], axis=1) * FP8_SCALE).astype(NPFP8)
            wv = np.ascontiguousarray(vcols).astype(NPBF16)
            m = dict(common, wqkv=wq8, wqv=wv,
                     wo=wo_halves[c // 4] if wo2d else wo_bf)
            maps.append(m)
        return maps
    common = {"hsT": hsT, "cos2": cos2, "sin2": sin2, "pmask": pmask}
    if wo2d:
        return [dict(common, wqkv=wqkv_shards[c], wo=wo_halves[c // 4])
                for c in range(NCORES)]
    return [dict(common, wqkv=wqkv_shards[c], wo=wo_bf) for c in range(NCORES)]


def _assemble(outs):
    full = np.empty((S, H), np.float32)
    for c in range(NCORES):
        g, ch = c % 4, c // 4
        full[g * 512:(g + 1) * 512, ch * (H // 2):(ch + 1) * (H // 2)] = outs[c]
    return full


def kernel(positions, hidden_states, Wqkv, Wo):
    in_maps = _host_prep(positions, hidden_states, Wqkv, Wo, wo2d=True)
    nc = _get_nc()
    res = run_bass_kernel_spmd(nc, in_maps, list(range(NCORES)))
    return _assemble([res.results[c]["out"] for c in range(NCORES)])

